# revision 19
# baseline (speedup 1.0000x reference)
# Trainium2 Bass kernel for BaseGumbelGraphNetwork message passing.
#
# Reference computation (B=4, N=512, D=2, H=64, O=2):
#   e1 = relu(cat(x_i, x_j) @ W_n2e.T + b_n2e)        [B,N,N,H]
#   e2 = relu(e1 @ W_e2e.T + b_e2e)                   [B,N,N,H]
#   s  = sum_j adj[i,j] * e2                          [B,N,H]
#   h  = relu(relu(s@W_e2n.T+b)@W_n2n.T+b)
#   out= relu(cat(x,h)@W_o1.T+b) @ W_o2.T + b         [B,N,O]
#
# Key structure: layer 1 factorizes over the (i,j) grid:
#   e1[b,i,j,:] = relu(A[b,i,:] + C[b,j,:] + b1),  A = x@Wi.T, C = x@Wj.T
# so the [B,N,N,2D] concat tensor is never materialized.
#
# Device layout (per core, i-dim sharded 8 ways -> 64 rows/core):
#   * a unit is (batch b, i-pair q): partitions = h stacked for the two i's
#     (2x64), free dim = j (512). Two consecutive q's share one [128,1024]
#     tile so ACT/DVE instruction overheads amortize. Loop: b outer, q inner
#     -- each batch's tiny output MLP overlaps the next batch's main loop.
#   * per (b, q-pair) iteration:
#       2x DVE tensor_scalar  e1 = relu(C.T + (A_i + b1))   fp16
#       2x PE matmul          e2pre = blockdiag(W_e2e.T).T @ e1  (fp16, N=512)
#       1x ACT                e2 = relu(e2pre + b2)  PSUM -> SBUF bf16 [128,1024]
#       1x DVE tensor_tensor  scr = e2 * adj_bcast             [128,1024] bf16
#       2x PE matmul (fused reduce): h1pre[:, b, q] = sum_j W_e2n_bd.T @ scr
#          via a PSUM output AP repeating 8 columns 64 times -- PSUM's
#          per-element has_written accumulate sums all 512 j-columns in
#          hardware (8-column spacing avoids the same-address RMW hazard).
#   * emission is software-pipelined (TT one iteration behind its ACT,
#     reduce matmuls two behind) since each engine runs its stream in order.
#   * adj rows are partition-broadcast into SBUF by DMA once and stay
#     resident for all four batches.
#   * startup: all weights/inputs arrive in 4 packed DMAs split across the
#     two HWDGE rings (sync + scalar).

import numpy as np

B, N, D, H, O = 4, 512, 2, 64, 2
NCORES = 8
IB = N // NCORES  # i rows per core = 64
Q = IB // 2       # i pairs per core = 32
T = Q // 2        # q-pair iterations per batch = 16

_STATE = {}

# wpack column layout (fp32, 128 partitions)
_WP = {}
_o = 0
for _name, _w in [("b1s", 1), ("b2s", 1), ("be2ns", 1), ("bn2ns", 1),
                  ("bo1s", 1), ("wn2nbd", 128), ("wo1hbd", 128),
                  ("wo2bd", 4), ("bo2s", 1)]:
    _WP[_name] = (_o, _o + _w)
    _o += _w
WPACK_COLS = _o

# xpk column layout (fp32, 4 partitions; per-b block after the fixed part)
XB = 512 + Q + Q + Q  # xT, xtie, xtio, xpair widths per b
XPK_FIX = 128 + 64 + 128  # wjt2, wit, wo1xbd
XPK_COLS = XPK_FIX + B * XB


def _build_nc():
    import concourse.mybir as mybir
    from concourse import bacc
    from concourse.tile import TileContext

    F32 = mybir.dt.float32
    FP16 = mybir.dt.float16   # e1 / W2-matmul path (better weight precision)
    BFL = mybir.dt.bfloat16   # e2 / mask / reduce path (full-rate ACT writes)
    AL = mybir.AluOpType
    AF = mybir.ActivationFunctionType

    nc = bacc.Bacc("TRN2", target_bir_lowering=False, debug=False,
                   num_devices=NCORES)

    def din(name, shape, dt=F32):
        return nc.dram_tensor(name, list(shape), dt, kind="ExternalInput").ap()

    wpack = din("wpack", (128, WPACK_COLS))
    xpk = din("xpk", (4, XPK_COLS))
    adjr = din("adjr", (IB, N), BFL)      # this core's adjacency row block
    w2bd = din("w2bd", (128, 128), FP16)  # blockdiag(W_e2e.T, W_e2e.T)
    we2nbd = din("we2nbd", (128, 128), BFL)  # blockdiag(W_e2n.T, W_e2n.T)

    out_d = nc.dram_tensor("out", [B, IB, O], F32, kind="ExternalOutput").ap()
    # out[b, 2q+e, o] <- OUT_sb[b][2e+o, q]
    out_re = out_d.rearrange("b (q e) o -> b (e o) q", e=2)

    with TileContext(nc) as tc:
        with (tc.tile_pool(name="wpool", bufs=1) as wp,
              tc.tile_pool(name="ctsp", bufs=B) as ctsp,
              tc.tile_pool(name="abp", bufs=B) as abp,
              tc.tile_pool(name="adjp", bufs=T) as adjp,
              tc.tile_pool(name="e1p", bufs=6) as e1p,
              tc.tile_pool(name="e2p", bufs=6) as e2p,
              tc.tile_pool(name="scrp", bufs=8) as scrp,
              tc.tile_pool(name="finp", bufs=2) as finp,
              tc.tile_pool(name="psp", bufs=3, space="PSUM") as psp,
              tc.tile_pool(name="hps", bufs=2, space="PSUM") as hps):

            # ---- packed loads: 2 on the scalar HWDGE ring, 2 on sync ----
            wpk = wp.tile([128, WPACK_COLS], F32, tag="wpk")
            nc.scalar.dma_start(out=wpk[:], in_=wpack[:])
            xpkt = wp.tile([4, XPK_COLS], F32, tag="xpk")
            nc.scalar.dma_start(out=xpkt[:], in_=xpk[:])
            w2bd_s = wp.tile([128, 128], FP16, tag="w2bd")
            nc.sync.dma_start(out=w2bd_s[:], in_=w2bd[:])
            we2nbd_s = wp.tile([128, 128], BFL, tag="we2nbd")
            nc.sync.dma_start(out=we2nbd_s[:], in_=we2nbd[:])

            def wslice(name):
                a, bb = _WP[name]
                return wpk[:, a:bb]
            b1s_s, b2s_s = wslice("b1s"), wslice("b2s")
            be2ns_s, bn2ns_s = wslice("be2ns"), wslice("bn2ns")
            bo1s_s = wslice("bo1s")
            wn2nbd_s, wo1hbd_s = wslice("wn2nbd"), wslice("wo1hbd")
            wo2bd_s = wslice("wo2bd")
            bo2s_s = wpk[0:4, _WP["bo2s"][0]:_WP["bo2s"][1]]
            wjt2_s = xpkt[0:2, 0:128]
            wit_s = xpkt[0:2, 128:192]
            wo1xbd_s = xpkt[0:4, 192:XPK_FIX]

            def xslice(b, off, w, rows=2):
                a = XPK_FIX + b * XB + off
                return xpkt[0:rows, a:a + w]

            # fused-reduce accumulators: h1pre 8-sub-columns per (b, q);
            # two PSUM banks, one per batch-pair
            h1ps = [hps.tile([128, 2 * Q * 8], F32, tag="h1ps",
                             name=f"h1ps{i}")
                    for i in range(2)]
            h1v = [t[:].rearrange("p (b q e) -> p b q e", b=2, e=8)
                   for t in h1ps]

            # ---- per-batch setup: CTS (stacked C.T) and ABIAS (A + b1) ----
            CTS, AB = [], []
            for b in range(B):
                ps = psp.tile([128, 1024], F32, tag="ps")
                nc.tensor.matmul(ps[:, 0:512], lhsT=wjt2_s,
                                 rhs=xslice(b, 0, 512), start=True, stop=True)
                cts = ctsp.tile([128, N], FP16, tag="cts")
                nc.scalar.copy(cts[:], ps[:, 0:512])

                ps2 = psp.tile([128, 1024], F32, tag="ps")
                nc.tensor.matmul(ps2[0:64, 0:Q], lhsT=wit_s,
                                 rhs=xslice(b, 512, Q), start=True, stop=True)
                nc.tensor.matmul(ps2[64:128, 0:Q], lhsT=wit_s,
                                 rhs=xslice(b, 512 + Q, Q),
                                 start=True, stop=True)
                ab = abp.tile([128, Q], F32, tag="ab")
                nc.vector.tensor_scalar_add(out=ab[:], in0=ps2[0:128, 0:Q],
                                            scalar1=b1s_s)
                CTS.append(cts)
                AB.append(ab)

            # ---- main loop: b outer, q-pair inner, software pipelined ----
            ADJ = [None] * T

            def emit_tt(jobs):
                for adjt_, e2m_, scrm_ in jobs:
                    nc.vector.tensor_tensor(
                        out=scrm_[:].rearrange("p (u j) -> p u j", u=2),
                        in0=e2m_[:].rearrange("p (u j) -> p u j", u=2),
                        in1=adjt_[:].rearrange("p (u j) -> p u j", u=2),
                        op=AL.mult)

            def emit_red(jobs):
                done_b = None
                for b, t, scrm_, k in jobs:
                    q = 2 * t + k
                    sl = h1v[b // 2][:, b % 2, q:q + 1, :]       # [128, 1, 8]
                    nc.tensor.matmul(sl.broadcast_to((128, 64, 8)),
                                     lhsT=we2nbd_s[:],
                                     rhs=scrm_[:, 512 * k:512 * (k + 1)],
                                     start=True, stop=True)
                    if t == T - 1 and k == 1:
                        done_b = b
                return done_b

            def emit_final(b):
                h1pre = finp.tile([128, Q], F32, tag="h1pre")
                nc.vector.tensor_reduce(out=h1pre[:],
                                        in_=h1v[b // 2][:, b % 2],
                                        axis=mybir.AxisListType.X, op=AL.add)
                h1 = finp.tile([128, Q], F32, tag="h1")
                nc.scalar.activation(h1[:], h1pre[:], AF.Relu, bias=be2ns_s)

                ps2 = psp.tile([128, 1024], F32, tag="ps")
                nc.tensor.matmul(ps2[:, 0:Q], lhsT=wn2nbd_s, rhs=h1[:],
                                 start=True, stop=True)
                h2 = finp.tile([128, Q], F32, tag="h2")
                nc.scalar.activation(h2[:], ps2[:, 0:Q], AF.Relu, bias=bn2ns_s)

                ps3 = psp.tile([128, 1024], F32, tag="ps")
                nc.tensor.matmul(ps3[:, 0:Q], lhsT=wo1hbd_s, rhs=h2[:],
                                 start=True, stop=False)
                nc.tensor.matmul(ps3[:, 0:Q], lhsT=wo1xbd_s,
                                 rhs=xslice(b, 512 + 2 * Q, Q, rows=4),
                                 start=False, stop=True)
                h3 = finp.tile([128, Q], F32, tag="h3")
                nc.scalar.activation(h3[:], ps3[:, 0:Q], AF.Relu, bias=bo1s_s)

                ps4 = psp.tile([128, 1024], F32, tag="ps")
                nc.tensor.matmul(ps4[0:4, 0:Q], lhsT=wo2bd_s, rhs=h3[:],
                                 start=True, stop=True)
                outs = finp.tile([4, Q], F32, tag="outs")
                nc.scalar.activation(outs[:], ps4[0:4, 0:Q], AF.Identity,
                                     bias=bo2s_s)
                nc.sync.dma_start(out=out_re[b], in_=outs[:])

            tt_prev, red_prev1, red_prev2 = [], [], []
            for b in range(B):
                for t in range(T):
                    if b == 0:
                        adjt = adjp.tile([128, 1024], BFL, tag="adj")
                        for r in range(4):
                            nc.sync.dma_start(
                                out=adjt[64 * (r % 2):64 * (r % 2 + 1),
                                         512 * (r // 2):512 * (r // 2 + 1)],
                                in_=adjr[4 * t + r:4 * t + r + 1, :]
                                .partition_broadcast(64))
                        ADJ[t] = adjt
                    adjt = ADJ[t]
                    e1m = e1p.tile([128, 1024], FP16, tag="e1")
                    for k in range(2):
                        q = 2 * t + k
                        nc.vector.tensor_scalar(
                            out=e1m[:, 512 * k:512 * (k + 1)], in0=CTS[b][:],
                            scalar1=AB[b][:, q:q + 1], scalar2=0.0,
                            op0=AL.add, op1=AL.max)
                    psm = psp.tile([128, 1024], F32, tag="ps")
                    nc.tensor.matmul(psm[:, 0:512], lhsT=w2bd_s[:],
                                     rhs=e1m[:, 0:512], start=True, stop=True)
                    nc.tensor.matmul(psm[:, 512:1024], lhsT=w2bd_s[:],
                                     rhs=e1m[:, 512:1024], start=True,
                                     stop=True)
                    e2m = e2p.tile([128, 1024], BFL, tag="e2")
                    nc.scalar.activation(e2m[:], psm[:], AF.Relu, bias=b2s_s)
                    scrm = scrp.tile([128, 1024], BFL, tag="scr")

                    emit_tt(tt_prev)
                    fb = emit_red(red_prev2)
                    tt_prev = [(adjt, e2m, scrm)]
                    red_prev2 = red_prev1
                    red_prev1 = [(b, t, scrm, 0), (b, t, scrm, 1)]
                    if fb is not None:
                        emit_final(fb)
            emit_tt(tt_prev)
            fb = emit_red(red_prev2)
            if fb is not None:
                emit_final(fb)
            fb = emit_red(red_prev1)
            if fb is not None:
                emit_final(fb)

    nc.compile()
    return nc


def _get_nc():
    if "nc" not in _STATE:
        _STATE["nc"] = _build_nc()
    return _STATE["nc"]


def _prep_maps(inputs):
    import ml_dtypes
    bfl = ml_dtypes.bfloat16
    fp16 = np.float16
    f32 = np.float32

    x = np.ascontiguousarray(np.asarray(inputs["input"], f32))      # [B,N,D]
    adj = np.ascontiguousarray(np.asarray(inputs["adj"], f32))      # [N,N]
    W_n2e = np.asarray(inputs["W_n2e"], f32)   # [H, 2D]
    b_n2e = np.asarray(inputs["b_n2e"], f32)
    W_e2e = np.asarray(inputs["W_e2e"], f32)
    b_e2e = np.asarray(inputs["b_e2e"], f32)
    W_e2n = np.asarray(inputs["W_e2n"], f32)
    b_e2n = np.asarray(inputs["b_e2n"], f32)
    W_n2n = np.asarray(inputs["W_n2n"], f32)
    b_n2n = np.asarray(inputs["b_n2n"], f32)
    W_o1 = np.asarray(inputs["W_o1"], f32)     # [H, D+H]
    b_o1 = np.asarray(inputs["b_o1"], f32)
    W_o2 = np.asarray(inputs["W_o2"], f32)     # [O, H]
    b_o2 = np.asarray(inputs["b_o2"], f32)

    Wi, Wj = W_n2e[:, :D], W_n2e[:, D:]

    def bd(w):  # blockdiag(w, w)
        r, c = w.shape
        z = np.zeros((2 * r, 2 * c), f32)
        z[:r, :c] = w
        z[r:, c:] = w
        return z

    wpack = np.zeros((128, WPACK_COLS), f32)

    def put(name, val, rows=128):
        a, bb = _WP[name]
        wpack[:rows, a:bb] = val
    put("b1s", np.concatenate([b_n2e, b_n2e]).reshape(128, 1))
    put("b2s", np.concatenate([b_e2e, b_e2e]).reshape(128, 1))
    put("be2ns", np.concatenate([b_e2n, b_e2n]).reshape(128, 1))
    put("bn2ns", np.concatenate([b_n2n, b_n2n]).reshape(128, 1))
    put("bo1s", np.concatenate([b_o1, b_o1]).reshape(128, 1))
    put("wn2nbd", bd(W_n2n.T))
    put("wo1hbd", bd(W_o1[:, D:].T))
    put("wo2bd", bd(W_o2.T))
    put("bo2s", np.concatenate([b_o2, b_o2]).reshape(4, 1), rows=4)

    maps = []
    for c in range(NCORES):
        sl = slice(c * IB, (c + 1) * IB)
        xc = x[:, sl]                                    # [B, IB, D]
        xpk = np.zeros((4, XPK_COLS), f32)
        xpk[0:2, 0:128] = np.concatenate([Wj.T, Wj.T], axis=1)
        xpk[0:2, 128:192] = Wi.T
        xpk[0:4, 192:XPK_FIX] = bd(W_o1[:, :D].T)
        for b in range(B):
            a = XPK_FIX + b * XB
            xpk[0:2, a:a + 512] = x[b].T
            xpk[0:2, a + 512:a + 512 + Q] = xc[b, 0::2].T
            xpk[0:2, a + 512 + Q:a + 512 + 2 * Q] = xc[b, 1::2].T
            xpk[0:4, a + 512 + 2 * Q:a + 512 + 3 * Q] = \
                xc[b].reshape(Q, 2 * D).T                # rows e*2+d
        m = {
            "wpack": wpack,
            "xpk": xpk,
            "adjr": adj[sl].astype(bfl),
            "w2bd": bd(W_e2e.T).astype(fp16),
            "we2nbd": bd(W_e2n.T).astype(bfl),
        }
        maps.append({k: np.ascontiguousarray(v) for k, v in m.items()})
    return maps


def run(inputs, trace=False, **kw):
    from concourse.bass_utils import run_bass_kernel_spmd
    nc = _get_nc()
    maps = _prep_maps(inputs)
    res = run_bass_kernel_spmd(nc, maps, list(range(NCORES)), trace=trace, **kw)
    out = np.concatenate([res.results[c]["out"] for c in range(NCORES)], axis=1)
    return np.ascontiguousarray(out, dtype=np.float32), res


def kernel(**inputs):
    out, _ = run(inputs, trace=False)
    return out
